# revision 1
# baseline (speedup 1.0000x reference)
"""Trainium2 Bass kernel for the PixelRNN Diagonal BiLSTM problem.

Contract: kernel(**inputs) takes FULL unsharded inputs (keyed as in
setup_inputs) and returns the FULL (32, 3, 256, 32, 32) float32 output.
Internally: pure data-parallel over 8 NeuronCores (4 images each), weights
replicated, no collectives.

Per-core dataflow (partition dim is always the 128-wide hidden/gate dim):
  X    (128, 94*128) SBUF "diag" buffer, column = 128*d + 32*b + s
       holding the in-projected pixel at diagonal d, batch b, step s
       (zero where the skewed-grid cell is padding).
  scan 32 steps x 2 directions. Step t of a direction works on the active
       diagonal window d in [t, 93-t] (N = 4*(94-2t) columns):
         4 matmuls  Wi.T-block @ x-cols   (PSUM, start=True)
         4 matmuls  Wh.T-block @ h(t-1)   (PSUM accumulate)
         5 ACT ops  sigmoid(i,f,o)+bias, tanh(g)+bias, tanh(c)
         4 DVE ops  c = sf*c + si*tg ; h = so*tc (h written strided into
                    a padded per-direction H buffer, col = 128d+32b+t)
       The backward direction reads X via 3 constant-stride pieces
       (strides 129/128/127) implementing the per-diagonal step reversal.
  gather 66 strided DVE copies -> un (128, 2*4096), then 96 matmuls for the
       output projection, biased eviction, DMA to DRAM.

All index formulas are validated against reference.py in layout_sim.py.
"""
from contextlib import ExitStack

import numpy as np

import concourse.bass as bass
import concourse.tile as tile
from concourse.tile import add_dep_helper
from concourse import mybir
from concourse.bass_utils import run_bass_kernel_spmd

AF = mybir.ActivationFunctionType
F32 = mybir.dt.float32
BF16 = mybir.dt.bfloat16
F32R = mybir.dt.float32r

BS = 4            # batch shard per core
NCORES = 8
H = W = 32
HC = 128
D = 94            # number of diagonals
S = 32            # max steps per diagonal
NCOL = D * 128    # diag/H buffer columns

# matmul input mode: 'f32' (exact, 4 cyc/row), 'f32r' (1 cyc/row at N>=256),
# 'bf16' (1 cyc/row, storage halves)
MM_MODE = 'f32r'


def _ap(t, off, dims):
    """Free-dim-strided AP on 2D SBUF tile t: dims = [[stride, count], ...]."""
    a = t[:, :]
    return bass.AP(tensor=a.tensor, offset=a.offset + off, ap=[a.ap[0]] + dims)


def build(nc, mm_mode=MM_MODE):
    # storage dtype of matmul inputs; float32r = fp32 bits with hardware
    # rounding on write so the PE can use the fast (1 cyc/row) fp32r path
    st_dt = {'bf16': BF16, 'f32r': F32R, 'f32': F32}[mm_mode]

    def mm_ap(x):
        return x

    # ---------------- DRAM I/O ----------------
    # All 128-partition matmul weights in ONE tensor (single DMA — matmuls
    # have a single HW sync-wait slot, so minimize distinct DMA semaphores),
    # all f32 per-partition vectors (biases) in another.
    # wpack cols: WiT_f | WhT_f | WiT_b | WhT_b | out_wT_f | out_wT_b
    #             0:512   512:1024 1024:1536 1536:2048 2048:2816 2816:3584
    # vpack cols: bias_f(0:4) bias_b(4:8) out_b(8:14) in_proj_b(14:15)
    xT_d = nc.dram_tensor("xT", (3, BS * 1024), F32, kind="ExternalInput")
    ipw_d = nc.dram_tensor("in_projT", (3, HC), F32, kind="ExternalInput")
    wpack_d = nc.dram_tensor("wpack", (HC, 3584), st_dt, kind="ExternalInput")
    vpack_d = nc.dram_tensor("vpack", (HC, 15), F32, kind="ExternalInput")
    out_d = nc.dram_tensor("out", (6, HC, BS * 1024), F32, kind="ExternalOutput")

    with tile.TileContext(nc) as tc, ExitStack() as ctx:
        const = ctx.enter_context(tc.tile_pool(name="const", bufs=1))
        big = ctx.enter_context(tc.tile_pool(name="big", bufs=1))
        hpool = ctx.enter_context(tc.tile_pool(name="hpool", bufs=1))
        etmp = ctx.enter_context(tc.tile_pool(name="etmp", bufs=2))
        ev = ctx.enter_context(tc.tile_pool(name="ev", bufs=2))
        psum = ctx.enter_context(tc.tile_pool(name="psum", bufs=1, space="PSUM"))

        # ---------------- load constants ----------------
        final_insts = []   # one NOP sync-dep each at kernel end (drain diet)
        ipw = const.tile([3, HC], F32)
        final_insts.append(nc.sync.dma_start(ipw, ipw_d.ap()))
        xT = const.tile([3, BS * 1024], F32)
        final_insts.append(nc.sync.dma_start(xT, xT_d.ap()))
        wpk = const.tile([HC, 3584], st_dt)
        final_insts.append(nc.sync.dma_start(wpk, wpack_d.ap()))
        vpk = const.tile([HC, 15], F32)
        final_insts.append(nc.sync.dma_start(vpk, vpack_d.ap()))
        wi = {'f': wpk[:, 0:512], 'b': wpk[:, 1024:1536]}
        wh = {'f': wpk[:, 512:1024], 'b': wpk[:, 1536:2048]}
        owf = wpk[:, 2048:2816]
        owb = wpk[:, 2816:3584]
        bias = {'f': vpk[:, 0:4], 'b': vpk[:, 4:8]}
        ob = vpk[:, 8:14]
        ipb = vpk[:, 14:15]

        # ---------------- big buffers ----------------
        X = big.tile([HC, NCOL], st_dt, tag="xun")
        Hbuf = {'f': hpool.tile([HC, NCOL], st_dt, name="Hf"),
                'b': hpool.tile([HC, NCOL], st_dt, name="Hb")}
        C = {'f': const.tile([HC, 4 * D], F32, name="Cf"),
             'b': const.tile([HC, 4 * D], F32, name="Cb")}

        U32 = mybir.dt.uint32
        nc.gpsimd.memset(X.bitcast(U32), 0)
        nc.gpsimd.memset(Hbuf['f'].bitcast(U32), 0)
        final_insts.append(nc.gpsimd.memset(Hbuf['b'].bitcast(U32), 0))

        # Per-engine semaphore consumers: matmuls have ONE HW sync-wait slot
        # (ACT/DVE have few), so each engine pre-observes the constant-load
        # DMA + memset semaphores via tiny ops with a single wait each.
        # After these, real instructions only ever wait on one producer.
        # Hb col 12031 = (d=93, s=31): beyond that diagonal's length, never
        # read or written by scan/gather -> safe for consumers to touch.
        hb_nc = Hbuf['b'][:, NCOL - 1:NCOL]
        if mm_mode == 'f32r':
            hb_nc = hb_nc.bitcast(F32)
        pdum = psum.tile([HC, 4, 512], F32, tag="Pf", name="pdum")
        wpk1 = wpk[:, 0:1].bitcast(F32) if mm_mode == 'f32r' else wpk[:, 0:1]
        nc.tensor.matmul(pdum[0:1, 0, 0:1], wpk1, wpk1,
                         start=True, stop=True)          # wpack DMA
        nc.tensor.matmul(pdum[0:1, 0, 0:1], ipw[:, 0:1], ipw[:, 0:1],
                         start=True, stop=True)          # ipw DMA
        nc.tensor.matmul(pdum[0:1, 0, 0:1], xT[:, 0:1], xT[:, 0:1],
                         start=True, stop=True)          # xT DMA
        nc.tensor.matmul(pdum[0:1, 0, 0:1], hb_nc, hb_nc,
                         start=True, stop=True)          # gpsimd memsets
        trash_a = const.tile([HC, 3], F32)
        trash_d = const.tile([HC, 20], F32)
        nc.scalar.activation(trash_a[:, 0:1], vpk[:, 0:1], AF.Copy)  # vpack DMA
        nc.scalar.activation(trash_a[:, 1:2], hb_nc, AF.Copy)        # memsets
        nc.vector.tensor_copy(trash_d[:, 0:1], vpk[:, 0:1])
        nc.vector.tensor_copy(trash_d[:, 1:2], hb_nc)
        nc.vector.tensor_copy(trash_d[0:1, 2:3], pdum[0:1, 0, 0:1])  # PE sem
        tc.no_sync_barrier()   # keep the consumers scheduled first

        # ---------------- phase 1: in_proj + scatter ----------------
        with nc.named_scope("in_proj"):
            pin = {0: psum.tile([HC, 4, 512], F32, tag="Pf", name="pin0"),
                   1: psum.tile([HC, 4, 512], F32, tag="Pb", name="pin1")}
            for b in range(BS):
                for rh in range(2):
                    c0 = (b * 2 + rh) * 512
                    nc.tensor.matmul(pin[rh][:, b, :], ipw, xT[:, c0:c0 + 512],
                                     start=True, stop=True)
            # ACT observes the in_proj matmul ticks once, so the scatter
            # ops below never need a PE wait on top of their self-ordering.
            nc.scalar.activation(trash_a[0:1, 2:3], pin[1][0:1, 3, 511:512],
                                 AF.Copy)
            # scatter rows 0..15 (one strided op, case A everywhere):
            # X col = 257r + 128j + 32b, in psum col = 512b + 32r + j
            src = pin[0][:, :, :].rearrange("p b (r j) -> p b r j", r=16)
            dst = _ap(X, 0, [[32, BS], [257, 16], [128, 32]])
            nc.scalar.activation(dst, src, AF.Identity, bias=ipb)
            # rows 16..31: per row, case A (stride 128) + case B (stride 127)
            for r in range(16, 32):
                na = 63 - 2 * r   # case A count (j <= 62-2r)
                ra = r - 16
                srcA = _ap(pin[1], ra * 32, [[512, BS], [1, na]])
                dstA = _ap(X, 257 * r, [[32, BS], [128, na]])
                nc.scalar.activation(dstA, srcA, AF.Identity, bias=ipb)
                nb = 32 - na      # case B count
                srcB = _ap(pin[1], ra * 32 + na, [[512, BS], [1, nb]])
                dstB = _ap(X, 255 * r + 127 * na + 62, [[32, BS], [127, nb]])
                nc.scalar.activation(dstB, srcB, AF.Identity, bias=ipb)

        # ---------------- phase 2: the two scans ----------------
        # X col written by the last in_proj scatter op (r=31 case B)
        obs_src = {'f': X[:, 8094:8095], 'b': X[:, 8094:8095]}
        with nc.named_scope("scan"):
            for t in range(S):
                nd = D - 2 * t
                N = BS * nd
                for dr in 'fb':
                    P = psum.tile([HC, 4, 512], F32, tag='P' + dr,
                                  name=f"P{dr}{t}")
                    # PE pre-observes the ACT tick that frees this PSUM slot
                    # (a 1-col garbage ldweights; the next matmul reloads its
                    # own weights) so gate matmuls keep a single sync wait.
                    ldw = nc.tensor.ldweights(obs_src[dr].bitcast(BF16))
                    first_mm = None
                    # ---- gate matmuls ----
                    if dr == 'f':
                        xpieces = [(129 * t, 128, nd)]
                    else:
                        xpieces = []
                        if t <= 30:
                            xpieces.append((128 * t, 129, 31 - t))
                        xpieces.append((128 * 31 + 31 - t, 128, 32))
                        if t <= 30:
                            xpieces.append((127 * 63 + 93 - t, 127, 31 - t))
                    for g in range(4):
                        wig = wi[dr][:, g * HC:(g + 1) * HC]
                        base = 0
                        for pi, (off, strd, cnt) in enumerate(xpieces):
                            rhs = _ap(X, off, [[strd, cnt], [32, BS]])
                            last_piece = pi == len(xpieces) - 1
                            mmi = nc.tensor.matmul(
                                P[:, g, base:base + BS * cnt], mm_ap(wig),
                                mm_ap(rhs), start=(pi == 0),
                                stop=(t == 0 and last_piece))
                            if first_mm is None:
                                first_mm = mmi
                                add_dep_helper(mmi.ins, ldw.ins, sync=False,
                                               reason="ldw observer first")
                            base += BS * cnt
                        if t > 0:
                            whg = wh[dr][:, g * HC:(g + 1) * HC]
                            rhs = _ap(Hbuf[dr], 129 * t - 1,
                                      [[128, nd], [32, BS]])
                            nc.tensor.matmul(P[:, g, 0:N], mm_ap(whg),
                                             mm_ap(rhs), start=False,
                                             stop=True)
                    # ---- activations ----
                    sf = etmp.tile([HC, N], F32, tag="sf", name=f"sf{dr}{t}")
                    nc.scalar.activation(sf, P[:, 1, 0:N], AF.Sigmoid,
                                         bias=bias[dr][:, 1:2])
                    si = etmp.tile([HC, N], F32, tag="si", name=f"si{dr}{t}")
                    nc.scalar.activation(si, P[:, 0, 0:N], AF.Sigmoid,
                                         bias=bias[dr][:, 0:1])
                    tg = etmp.tile([HC, N], F32, tag="tg", name=f"tg{dr}{t}")
                    nc.scalar.activation(tg, P[:, 3, 0:N], AF.Tanh,
                                         bias=bias[dr][:, 3:4])
                    so = etmp.tile([HC, N], F32, tag="so", name=f"so{dr}{t}")
                    nc.scalar.activation(so, P[:, 2, 0:N], AF.Sigmoid,
                                         bias=bias[dr][:, 2:3])
                    # ---- cell/hidden update ----
                    c_sl = C[dr][:, 4 * t:4 * t + N]
                    if t == 0:
                        nc.vector.tensor_mul(c_sl, si, tg)
                    else:
                        # DVE observes its own previous-step c-write tick
                        # (both waits land on the DVE proc = one wait) so the
                        # in-place c multiply below only waits on ACT.
                        tch = nc.vector.tensor_copy(
                            trash_d[0:1, 4 + ('fb'.index(dr)):
                                    5 + ('fb'.index(dr))],
                            last_h[0][0:1, last_h[1]:last_h[1] + 1]
                            .bitcast(F32) if mm_mode == 'f32r' else
                            last_h[0][0:1, last_h[1]:last_h[1] + 1])
                        cm = nc.vector.tensor_mul(c_sl, c_sl, sf)
                        add_dep_helper(cm.ins, tch.ins, sync=False,
                                       reason="c touch first")
                        nc.vector.tensor_mul(si, si, tg)
                        nc.vector.tensor_add(c_sl, c_sl, si)
                    tc_ = etmp.tile([HC, N], F32, tag="tc", name=f"tc{dr}{t}")
                    act_i = nc.scalar.activation(tc_, c_sl, AF.Tanh)
                    hdst = _ap(Hbuf[dr], 129 * t, [[128, nd], [32, BS]])
                    nc.vector.tensor_mul(hdst, so, tc_)
                    last_h = (Hbuf[dr], 129 * t)
                    obs_src[dr] = tc_[:, 0:1]
                    last_tc = tc_

        # ---------------- phase 3: gather un ----------------
        with nc.named_scope("gather"):
            un = big.tile([HC, 2 * BS * 1024], st_dt, tag="xun")
            # fwd rows 0..15: one op
            nc.vector.tensor_copy(
                _ap(un, 0, [[1024, BS], [32, 16], [1, 32]]),
                _ap(Hbuf['f'], 0, [[32, BS], [257, 16], [128, 32]]))
            # fwd rows 16..31: A stride 128 (j<=62-2r), B stride 127
            for r in range(16, 32):
                na = 63 - 2 * r
                nb = 32 - na
                nc.vector.tensor_copy(
                    _ap(un, r * 32, [[1024, BS], [1, na]]),
                    _ap(Hbuf['f'], 257 * r, [[32, BS], [128, na]]))
                nc.vector.tensor_copy(
                    _ap(un, r * 32 + na, [[1024, BS], [1, nb]]),
                    _ap(Hbuf['f'], 255 * r + 127 * na + 62,
                        [[32, BS], [127, nb]]))
            # bwd rows 16..31: one op (col = 255r + 128j + 32b + 31)
            nc.vector.tensor_copy(
                _ap(un, 4096 + 16 * 32, [[1024, BS], [32, 16], [1, 32]]),
                _ap(Hbuf['b'], 255 * 16 + 31, [[32, BS], [255, 16], [128, 32]]))
            # bwd rows 0..15: A' stride 129 (j<=30-2r), B' stride 128
            for r in range(16):
                na = 31 - 2 * r
                nb = 32 - na
                nc.vector.tensor_copy(
                    _ap(un, 4096 + r * 32, [[1024, BS], [1, na]]),
                    _ap(Hbuf['b'], 257 * r, [[32, BS], [129, na]]))
                nc.vector.tensor_copy(
                    _ap(un, 4096 + r * 32 + na, [[1024, BS], [1, nb]]),
                    _ap(Hbuf['b'], 255 * r + 128 * na + 31,
                        [[32, BS], [128, nb]]))

        # ---------------- phase 4: output projection ----------------
        # Eviction targets reuse the (now dead) H buffers so no SBUF slot
        # rotates under a pending DMA: 48 chunks of 512 cols -> evA (23),
        # evB (23), evx (2). Every instruction keeps <=1 sync wait:
        #   ldweights observers give PE the ACT/DVE ticks it needs,
        #   a DVE copy of the last matmul output gives DVE the PE tick.
        with nc.named_scope("out_proj"):
            ldw_a = nc.tensor.ldweights(last_tc[:, 0:1].bitcast(BF16))  # ACT
            ldw_b = nc.tensor.ldweights(un[:, 4577:4578].bitcast(BF16))  # DVE
            evA = hpool.tile([HC, NCOL], F32, tag="Hf", name="evA")
            evB = hpool.tile([HC, NCOL], F32, tag="Hb", name="evB")
            evx = ev.tile([HC, 1024], F32, name="evx")

            def chunk_dst(q):
                if q < 23:
                    return evA[:, 512 * q:512 * (q + 1)]
                if q < 46:
                    return evB[:, 512 * (q - 23):512 * (q - 22)]
                return evx[:, 512 * (q - 46):512 * (q - 45)]

            last_ev = None
            nobs = 0
            for m in range(6):
                for half in range(2):
                    P = psum.tile([HC, 4, 512], F32, tag='P' + 'fb'[half],
                                  name=f"Po{m}{half}")
                    ldw2 = None
                    if last_ev is not None:
                        # PE observes the DVE evict ticks freeing this slot
                        ldw2 = nc.tensor.ldweights(last_ev.bitcast(BF16))
                    for cb in range(4):
                        ch = half * 4 + cb
                        mmi = nc.tensor.matmul(
                            P[:, cb, :], mm_ap(owf[:, m * HC:(m + 1) * HC]),
                            mm_ap(un[:, ch * 512:(ch + 1) * 512]),
                            start=True, stop=False)
                        last_mm = mmi
                        if cb == 0 and ldw2 is not None:
                            add_dep_helper(mmi.ins, ldw2.ins, sync=False,
                                           reason="ldw observer first")
                        if cb == 0 and m == 0 and half == 0:
                            add_dep_helper(mmi.ins, ldw_a.ins, sync=False,
                                           reason="initial observers first")
                            add_dep_helper(mmi.ins, ldw_b.ins, sync=False,
                                           reason="initial observers first")
                        last_mm = nc.tensor.matmul(
                            P[:, cb, :], mm_ap(owb[:, m * HC:(m + 1) * HC]),
                            mm_ap(un[:, 4096 + ch * 512:4096 + (ch + 1) * 512]),
                            start=False, stop=True)
                    # DVE observes this group's matmul ticks (reads the last
                    # matmul's PSUM output) so evicts carry no PE wait.
                    obs = nc.vector.tensor_copy(trash_d[0:1, 6 + nobs:
                                                        7 + nobs],
                                                P[0:1, 3, 511:512])
                    nobs += 1
                    for cb in range(4):
                        q = m * 8 + half * 4 + cb
                        dst = chunk_dst(q)
                        evi = nc.vector.tensor_scalar_add(dst, P[:, cb, :],
                                                          ob[:, m:m + 1])
                        last_evi = evi
                        if cb == 0:
                            add_dep_helper(evi.ins, obs.ins, sync=False,
                                           reason="dve observer first")
                        last_ev = dst[:, 0:1]
                # DMA this m's 4096 cols out (split where buffers change)
                runs = []
                q0 = m * 8
                for q in range(q0, q0 + 8):
                    buf = 0 if q < 23 else (1 if q < 46 else 2)
                    if runs and runs[-1][0] == buf:
                        runs[-1][2] += 512
                    else:
                        runs.append([buf, q, 512])
                col = 0
                for buf, qs, width in runs:
                    srcs = {0: evA, 1: evB, 2: evx}[buf]
                    off = {0: 512 * qs, 1: 512 * (qs - 23), 2: 512 * (qs - 46)}[buf]
                    # SWDGE (gpsimd) queues are unused so far: each of the 8
                    # output DMAs gets a fresh lane -> no queue-FIFO wait on
                    # top of its single DVE data wait.
                    final_insts.append(
                        nc.gpsimd.dma_start(out_d.ap()[m, :, col:col + width],
                                            srcs[:, off:off + width]))
                    col += width
            # tail sweep: SP observes every proc's final tick via 1-wait
            # NOPs so the TileContext exit drain needs no waits of its own.
            final_insts += [act_i, last_mm, last_evi]
            for fi in final_insts:
                nop = nc.sync.nop()
                add_dep_helper(nop.ins, fi.ins, sync=True,
                               reason="drain diet: pre-observe final ticks")
    return nc


def _prep_inputs(inputs, mm_mode=MM_MODE):
    """Host-side weight reshaping -> per-core in_maps."""
    st_np = np.float32 if mm_mode != 'bf16' else None  # bf16 handled below
    import ml_dtypes
    bf = ml_dtypes.bfloat16

    def cast(a):
        a = np.ascontiguousarray(a, np.float32)
        return a.astype(bf) if mm_mode == 'bf16' else a

    x = np.asarray(inputs['x'], np.float32)
    wpack = np.concatenate([
        np.asarray(inputs['fwd_Wi'], np.float32).T,
        np.asarray(inputs['fwd_Wh'], np.float32).T,
        np.asarray(inputs['bwd_Wi'], np.float32).T,
        np.asarray(inputs['bwd_Wh'], np.float32).T,
        np.asarray(inputs['out_w'], np.float32)[:, :HC].T,
        np.asarray(inputs['out_w'], np.float32)[:, HC:].T,
    ], axis=1)                                             # (128, 3584)
    vpack = np.concatenate([
        np.asarray(inputs['fwd_b'], np.float32).reshape(4, HC).T,
        np.asarray(inputs['bwd_b'], np.float32).reshape(4, HC).T,
        np.asarray(inputs['out_b'], np.float32).reshape(6, HC).T,
        np.asarray(inputs['in_proj_b'], np.float32).reshape(HC, 1),
    ], axis=1)                                             # (128, 15)
    common = {
        "in_projT": np.ascontiguousarray((np.asarray(inputs['in_proj_w']) / 255.0).T,
                                         np.float32),
        "wpack": cast(wpack),
        "vpack": np.ascontiguousarray(vpack),
    }
    in_maps = []
    for c in range(NCORES):
        xs = x[c * BS:(c + 1) * BS]                    # (4, 3, 32, 32)
        xT = np.ascontiguousarray(
            xs.transpose(1, 0, 2, 3).reshape(3, BS * 1024))
        in_maps.append({"xT": xT, **common})
    return in_maps


def _assemble(results):
    outs = []
    for r in results:
        lg = r["out"]                                   # (6, 128, 4096)
        lg = lg.reshape(6, HC, BS, H, W).transpose(2, 0, 1, 3, 4)
        outs.append(lg.reshape(BS, 768, H, W))
    full = np.concatenate(outs, axis=0)                 # (32, 768, 32, 32)
    return np.ascontiguousarray(
        full.reshape(32, 3, 256, H, W).astype(np.float32))


def kernel(**inputs):
    nc = bass.Bass("TRN2", target_bir_lowering=False, debug=False)
    build(nc, MM_MODE)
    in_maps = _prep_inputs(inputs, MM_MODE)
    res = run_bass_kernel_spmd(nc, in_maps, core_ids=list(range(NCORES)))
    return _assemble(res.results)


if __name__ == "__main__":
    # quick IR build smoke test (no hardware)
    nc = bass.Bass("TRN2", target_bir_lowering=False, debug=False)
    build(nc, MM_MODE)
    print("IR build OK")

